# revision 15
# baseline (speedup 1.0000x reference)
"""Causal self-attention (B=2, T=2048, C=1024, H=16) on 8 TRN2 NeuronCores.

Sharding: core = (batch b, head-group g); b = core // 4, g = core % 4.
Each core computes Q/K/V projections for its 4 heads (column-sharded
Wq/Wk/Wv), causal attention for those heads, and a partial output
projection (row-sharded Wo). Host sums the 4 partials per batch (the
"all-reduce") and adds bo. No FLOP duplication across cores.

On-chip layout (per core; MM_DT is the matmul dtype, bf16 by default):
  xT   [c=128*8, t=2048]  x transposed via PE (projections contract over c)
  qt/kt [dq=128*2, t]     Q^T/K^T; head h lives in pblock h//2, rows 64*(h%2)
  v    [t-blocks][128, 4*65]  per-head V columns + a ones column (fused rowsum)
  scores S^T [k=128, q=512] = K^T.T @ Q^T per (head, q-tile, k-block)
  P~ = exp(S^T/8 + causal_mask)  (ACT, fused scale; additive mask only on
       diagonal blocks, sliced from a [128,512] master tile)
  O^T+rowsum [65, q] = [V | 1].T @ P~  accumulated over k-blocks (PSUM)
  normalize: recip(rowsum) broadcast across partitions via PE outer product,
       multiplied in during the PSUM->SBUF move
  Y_partial [t, c] = O^T.T @ Wo_rows  -> DRAM
"""
import os
import sys

sys.path.insert(0, "/opt/trn_rl_repo")

import numpy as np

B, T, C, H = 2, 2048, 1024, 16
HD = C // H            # 64
N_CORES = 8
GROUPS = 4             # head-groups per batch -> 4 cores per batch
HPG = H // GROUPS      # heads per core = 4
DHG = HPG * HD         # head-dim per core = 256
NEG = -1.0e9
TB = T // 128          # 16 t-blocks
CB = C // 128          # 8 c-blocks
NQT = T // 512         # 4 q-tiles
VW = HPG * (HD + 1)    # 260 v columns per t-block

# matmul dtype: "bf16" (fast, ~2e-3 rel err) or "f32r" (~2.5e-4, ~2x slower)
MM_MODE = os.environ.get("ATTN_MM_DT", "bf16")

_CACHE = {}


def _build():
    from concourse import bacc
    import concourse.mybir as mybir
    from concourse.tile import TileContext
    from concourse.masks import make_identity

    f32 = mybir.dt.float32
    mdt = mybir.dt.bfloat16 if MM_MODE == "bf16" else mybir.dt.float32r
    FT = mybir.ActivationFunctionType

    nc = bacc.Bacc(trn_type="TRN2", target_bir_lowering=False, debug=False,
                   num_devices=N_CORES)
    xd = nc.dram_tensor("x", [T, C], f32, kind="ExternalInput").ap()
    wqd = nc.dram_tensor("wq", [C, DHG], f32, kind="ExternalInput").ap()
    wkd = nc.dram_tensor("wk", [C, DHG], f32, kind="ExternalInput").ap()
    wvd = nc.dram_tensor("wv", [C, DHG], f32, kind="ExternalInput").ap()
    wod = nc.dram_tensor("wo", [DHG, C], f32, kind="ExternalInput").ap()
    bqd = nc.dram_tensor("bq", [DHG, 1], f32, kind="ExternalInput").ap()
    bkd = nc.dram_tensor("bk", [DHG, 1], f32, kind="ExternalInput").ap()
    bvd = nc.dram_tensor("bv", [1, DHG], f32, kind="ExternalInput").ap()
    mmd = nc.dram_tensor("maskm", [128, 512], f32, kind="ExternalInput").ap()
    yd = nc.dram_tensor("y", [T, C], f32, kind="ExternalOutput").ap()

    with TileContext(nc) as tc:
        with tc.tile_pool(name="persist", bufs=1) as pp:
            qt_all = pp.tile([128, 2 * T], mdt, tag="qt")
            kt_all = pp.tile([128, 2 * T], mdt, tag="kt")
            v_all = pp.tile([128, TB * VW], mdt, tag="v")
            ot_all = pp.tile([128, 2 * T], mdt, tag="ot")
            wo_t = pp.tile([128, 2 * C], mdt, tag="wo")
            maskm = pp.tile([128, 512], f32, tag="mm")
            ident = pp.tile([128, 128], mdt, tag="id")
            ones64 = pp.tile([1, HD], mdt, tag="o64")
            onest = pp.tile([1, 128], mdt, tag="ot1")
            bq_c = pp.tile([128, 2], f32, tag="bqc")
            bk_c = pp.tile([128, 2], f32, tag="bkc")
            bv_r = pp.tile([1, DHG], mdt, tag="bvr")
            zstage = pp.tile([128, 128], f32, tag="zst")
            zrow = pp.tile([128, 384], mdt, tag="zrow")

            nc.sync.dma_start(out=maskm[:], in_=mmd)
            make_identity(nc, ident[:])
            # memset can't produce bf16/f32r reliably across walrus builds;
            # build ones as 0 + 1.0 on ScalarE
            nc.gpsimd.memset(zstage[:], 0.0)
            nc.scalar.add(ones64[:], zstage[0:1, 0:HD], 1.0)
            for j in range(3):
                nc.scalar.add(zrow[:, j * 128:(j + 1) * 128], zstage[:], 0.0)
            nc.scalar.add(onest[:], zstage[0:1, :], 1.0)
            # ones columns for the fused rowsum (strided view, one col per
            # (t-block, head) section); data columns are overwritten by the
            # V-projection copies below.
            v4 = v_all[:].rearrange("p (t h c) -> p t h c", h=HPG, c=HD + 1)
            z4 = zstage[:, 0:TB * HPG].rearrange("p (t h) -> p t h", h=HPG).unsqueeze(3)
            nc.scalar.add(v4[:, :, :, HD:HD + 1], z4, 1.0)
            for pb in range(2):
                nc.sync.dma_start(out=bq_c[:, pb:pb + 1],
                                  in_=bqd[pb * 128:(pb + 1) * 128, :])
                nc.sync.dma_start(out=bk_c[:, pb:pb + 1],
                                  in_=bkd[pb * 128:(pb + 1) * 128, :])
            nc.gpsimd.dma_start(out=bv_r[:], in_=bvd)

            # ---- P1+P2: x load/transpose and Q/K/V projections ----
            with tc.tile_pool(name="ph12", bufs=1) as p12:
                xT = p12.tile([128, CB * T], mdt, tag="xT")
                wq_t = p12.tile([128, CB * DHG], mdt, tag="wqt")
                wk_t = p12.tile([128, CB * DHG], mdt, tag="wkt")
                wv_t = p12.tile([128, CB * DHG], mdt, tag="wvt")
                # weights ride the sync (HWDGE) queue in fp32 while x rides
                # gpsimd (casting SWDGE); DVE casts staging -> bf16
                with tc.tile_pool(name="wstage", bufs=1) as ws:
                    wq_s = ws.tile([128, CB * DHG], f32, tag="wqs")
                    wk_s = ws.tile([128, CB * DHG], f32, tag="wks")
                    wv_s = ws.tile([128, CB * DHG], f32, tag="wvs")
                    wo_s = ws.tile([128, C * 2], f32, tag="wos")
                    for cb in range(CB):
                        nc.sync.dma_start(out=wq_s[:, cb * DHG:(cb + 1) * DHG],
                                          in_=wqd[cb * 128:(cb + 1) * 128, :])
                        nc.sync.dma_start(out=wk_s[:, cb * DHG:(cb + 1) * DHG],
                                          in_=wkd[cb * 128:(cb + 1) * 128, :])
                        nc.sync.dma_start(out=wv_s[:, cb * DHG:(cb + 1) * DHG],
                                          in_=wvd[cb * 128:(cb + 1) * 128, :])
                    for pb in range(2):
                        nc.sync.dma_start(out=wo_s[:, pb * C:(pb + 1) * C],
                                          in_=wod[pb * 128:(pb + 1) * 128, :])
                    nc.scalar.copy(wq_t[:], wq_s[:])
                    nc.scalar.copy(wk_t[:], wk_s[:])
                    nc.scalar.copy(wv_t[:], wv_s[:])
                    nc.scalar.copy(wo_t[:], wo_s[:])

                with (tc.tile_pool(name="xstage", bufs=3) as xs,
                      tc.tile_pool(name="pstr", bufs=4, space="PSUM") as pst):
                    for tb in range(TB):
                        # gpsimd DMA casts fp32 -> mdt on the way in
                        xrow = xs.tile([128, C], mdt)
                        nc.gpsimd.dma_start(out=xrow[:],
                                            in_=xd[tb * 128:(tb + 1) * 128, :])
                        for cb in range(CB):
                            ptr = pst.tile([128, 128], mdt)
                            nc.tensor.transpose(ptr[:], xrow[:, cb * 128:(cb + 1) * 128],
                                                ident[:])
                            dst = xT[:, cb * T + tb * 128: cb * T + (tb + 1) * 128]
                            nc.vector.tensor_copy(out=dst, in_=ptr[:])

                with tc.tile_pool(name="psqk", bufs=3, space="PSUM") as pqk:
                    # tt-outer, V interleaved, so the first q/k/v stripes
                    # finish first and P3 can start while later stripes
                    # are still projecting
                    for tt in range(NQT):
                        for wt, bias_c, dst_all in ((wq_t, bq_c, qt_all),
                                                    (wk_t, bk_c, kt_all)):
                            for pb in range(2):
                                ps = pqk.tile([128, 512], f32, tag="psqk")
                                for cb in range(CB):
                                    nc.tensor.matmul(
                                        ps[:],
                                        wt[:, cb * DHG + pb * 128: cb * DHG + (pb + 1) * 128],
                                        xT[:, cb * T + tt * 512: cb * T + (tt + 1) * 512],
                                        start=(cb == 0), stop=(cb == CB - 1))
                                nc.vector.tensor_scalar_add(
                                    out=dst_all[:, pb * T + tt * 512: pb * T + (tt + 1) * 512],
                                    in0=ps[:], scalar1=bias_c[:, pb:pb + 1])
                        # V projection for this q-tile's t-blocks
                        for tb in range(4 * tt, 4 * tt + 4):
                            psv = pqk.tile([128, DHG], f32, tag="psv")
                            for cb in range(CB):
                                nc.tensor.matmul(
                                    psv[:],
                                    xT[:, cb * T + tb * 128: cb * T + (tb + 1) * 128],
                                    wv_t[:, cb * DHG:(cb + 1) * DHG],
                                    start=(cb == 0), stop=False)
                            nc.tensor.matmul(psv[:], onest[:], bv_r[:],
                                             start=False, stop=True)
                            for hh in range(HPG):
                                dst = v_all[:, tb * VW + hh * (HD + 1):
                                            tb * VW + hh * (HD + 1) + HD]
                                nc.vector.tensor_copy(out=dst,
                                                      in_=psv[:, hh * HD:(hh + 1) * HD])

            # ---- P3: attention; per (q-tile, k-block) all 4 heads are
            # emitted together so PE streams scores (alternating array
            # row-groups) while ACT exps trail one stage behind ----
            with (tc.tile_pool(name="ps_s", bufs=4, space="PSUM") as pss,
                  tc.tile_pool(name="ps_o", bufs=1, space="PSUM") as pso,
                  tc.tile_pool(name="sb3", bufs=6) as sb3):
                for qt in range(NQT):
                    nkb = 4 * (qt + 1)
                    po_l = [pso.tile([128, 512], f32, tag=f"po{h}", name=f"po{h}")
                            for h in range(HPG)]
                    for kb in range(nkb):
                        # cols [0, c) of this k-block's S^T are fully masked
                        # (q < k); skip them in the matmuls and exp, and
                        # zero them in P~ instead
                        c = max(0, (kb - 4 * qt) * 128)
                        sp_l = []
                        for h in range(HPG):
                            pbh, rh = h // 2, 64 * (h % 2)
                            sp = pss.tile([128, 512], f32, tag="sp")
                            nc.tensor.matmul(
                                sp[:, c:512],
                                kt_all[rh:rh + HD, pbh * T + kb * 128: pbh * T + (kb + 1) * 128],
                                qt_all[rh:rh + HD, pbh * T + qt * 512 + c: pbh * T + (qt + 1) * 512],
                                start=True, stop=True)
                            sp_l.append(sp)
                        for h in range(HPG):
                            sp = sp_l[h]
                            if kb >= 4 * qt:
                                nc.vector.tensor_add(out=sp[:, c:c + 128],
                                                     in0=sp[:, c:c + 128],
                                                     in1=maskm[:, 384:512])
                            pt = sb3.tile([128, 512], mdt, tag="pt")
                            if c > 0:
                                nc.vector.tensor_copy(out=pt[:, 0:c], in_=zrow[:, 0:c])
                            nc.scalar.activation(pt[:, c:512], sp[:, c:512], FT.Exp,
                                                 bias=0.0, scale=1.0 / np.sqrt(HD))
                            nc.tensor.matmul(
                                po_l[h][0:HD + 1, c:512],
                                v_all[:, kb * VW + h * (HD + 1): kb * VW + (h + 1) * (HD + 1)],
                                pt[:, c:512], start=(kb == 0), stop=(kb == nkb - 1))
                    for h in range(HPG):
                        pbh, rh = h // 2, 64 * (h % 2)
                        po = po_l[h]
                        rsum = sb3.tile([1, 512], f32, tag="rsum")
                        nc.vector.tensor_copy(out=rsum[:], in_=po[HD:HD + 1, :])
                        rr = sb3.tile([1, 512], f32, tag="rr")
                        nc.vector.reciprocal_approx_fast(out=rr[:], in_=rsum[:])
                        rrm = sb3.tile([1, 512], mdt, tag="rrm")
                        nc.vector.tensor_copy(out=rrm[:], in_=rr[:])
                        # broadcast recip into the unused partitions 64..127
                        # of this head's po bank (rsum row already consumed)
                        nc.tensor.matmul(po[HD:HD + 64, :], ones64[:], rrm[:],
                                         start=True, stop=True,
                                         tile_position=(0, HD))
                        rbs = sb3.tile([HD, 512], f32, tag="rbs")
                        nc.vector.tensor_copy(out=rbs[:], in_=po[HD:HD + 64, :])
                        nc.vector.tensor_mul(
                            out=ot_all[rh:rh + HD, pbh * T + qt * 512: pbh * T + (qt + 1) * 512],
                            in0=po[0:HD, :], in1=rbs[:])

            # ---- P4: output projection (partial; host sums across cores) ----
            with (tc.tile_pool(name="ps_y", bufs=3, space="PSUM") as psy,
                  tc.tile_pool(name="sb4", bufs=3) as sb4):
                for tb in range(TB):
                    for ct in range(2):
                        py = psy.tile([128, 512], f32, tag="py")
                        for pb in range(2):
                            nc.tensor.matmul(
                                py[:],
                                ot_all[:, pb * T + tb * 128: pb * T + (tb + 1) * 128],
                                wo_t[:, pb * C + ct * 512: pb * C + (ct + 1) * 512],
                                start=(pb == 0), stop=(pb == 1))
                        ys = sb4.tile([128, 512], f32, tag="ys")
                        nc.scalar.copy(ys[:], py[:])
                        nc.sync.dma_start(out=yd[tb * 128:(tb + 1) * 128,
                                                 ct * 512:(ct + 1) * 512],
                                          in_=ys[:])
    nc.finalize()
    return nc


def _get_nc():
    if "nc" not in _CACHE:
        _CACHE["nc"] = _build()
    return _CACHE["nc"]


def kernel(x, mask, Wq, bq, Wk, bk, Wv, bv, Wo, bo):
    from concourse import bass_utils

    x = np.ascontiguousarray(np.asarray(x, dtype=np.float32))
    Wq = np.asarray(Wq, dtype=np.float32)
    Wk = np.asarray(Wk, dtype=np.float32)
    Wv = np.asarray(Wv, dtype=np.float32)
    Wo = np.asarray(Wo, dtype=np.float32)
    bq = np.asarray(bq, dtype=np.float32)
    bk = np.asarray(bk, dtype=np.float32)
    bv = np.asarray(bv, dtype=np.float32)
    bo = np.asarray(bo, dtype=np.float32)

    m2 = np.asarray(mask).reshape(T, T)
    if not np.array_equal(m2, np.tril(np.ones((T, T), dtype=bool))):
        raise NotImplementedError("kernel is specialized to the causal tril mask")
    # additive mask master tile in S^T layout [k-row kk, col i]:
    # masked (q < k) iff (i - 384) < kk, where qq = i - (384 - c)
    ii = np.arange(512)[None, :]
    kk = np.arange(128)[:, None]
    maskm = np.ascontiguousarray(
        np.where((ii - 384) < kk, np.float32(NEG), np.float32(0.0)).astype(np.float32))

    nc = _get_nc()
    in_maps = []
    for core in range(N_CORES):
        b, g = core // GROUPS, core % GROUPS
        cols = slice(g * DHG, (g + 1) * DHG)
        in_maps.append({
            "x": np.ascontiguousarray(x[b]),
            "wq": np.ascontiguousarray(Wq[:, cols]),
            "wk": np.ascontiguousarray(Wk[:, cols]),
            "wv": np.ascontiguousarray(Wv[:, cols]),
            "wo": np.ascontiguousarray(Wo[cols, :]),
            "bq": np.ascontiguousarray(bq[cols].reshape(DHG, 1)),
            "bk": np.ascontiguousarray(bk[cols].reshape(DHG, 1)),
            "bv": np.ascontiguousarray(bv[cols].reshape(1, DHG)),
            "maskm": maskm,
        })

    trace = bool(int(os.environ.get("ATTN_TRACE", "0")))
    res = bass_utils.run_bass_kernel_spmd(nc, in_maps,
                                          core_ids=list(range(N_CORES)),
                                          trace=trace)
    _CACHE["last_result"] = res

    out = np.zeros((B, T, C), dtype=np.float32)
    for core in range(N_CORES):
        out[core // GROUPS] += res.results[core]["y"]
    out += bo
    return out


# revision 16
# speedup vs baseline: 1.0482x; 1.0482x over previous
"""Causal self-attention (B=2, T=2048, C=1024, H=16) on 8 TRN2 NeuronCores.

Sharding: core = (batch b, head-group g); b = core // 4, g = core % 4.
Each core computes Q/K/V projections for its 4 heads (column-sharded
Wq/Wk/Wv), causal attention for those heads, and a partial output
projection (row-sharded Wo). Host sums the 4 partials per batch (the
"all-reduce") and adds bo. No FLOP duplication across cores.

On-chip layout (per core; MM_DT is the matmul dtype, bf16 by default):
  xT   [c=128*8, t=2048]  x transposed via PE (projections contract over c)
  qt/kt [dq=128*2, t]     Q^T/K^T; head h lives in pblock h//2, rows 64*(h%2)
  v    [t-blocks][128, 4*65]  per-head V columns + a ones column (fused rowsum)
  scores S^T [k=128, q=512] = K^T.T @ Q^T per (head, q-tile, k-block)
  P~ = exp(S^T/8 + causal_mask)  (ACT, fused scale; additive mask only on
       diagonal blocks, sliced from a [128,512] master tile)
  O^T+rowsum [65, q] = [V | 1].T @ P~  accumulated over k-blocks (PSUM)
  normalize: recip(rowsum) broadcast across partitions via PE outer product,
       multiplied in during the PSUM->SBUF move
  Y_partial [t, c] = O^T.T @ Wo_rows  -> DRAM
"""
import os
import sys

sys.path.insert(0, "/opt/trn_rl_repo")

import numpy as np

B, T, C, H = 2, 2048, 1024, 16
HD = C // H            # 64
N_CORES = 8
GROUPS = 4             # head-groups per batch -> 4 cores per batch
HPG = H // GROUPS      # heads per core = 4
DHG = HPG * HD         # head-dim per core = 256
NEG = -1.0e9
TB = T // 128          # 16 t-blocks
CB = C // 128          # 8 c-blocks
NQT = T // 512         # 4 q-tiles
VW = HPG * (HD + 1)    # 260 v columns per t-block

# matmul dtype: "bf16" (fast, ~2e-3 rel err) or "f32r" (~2.5e-4, ~2x slower)
MM_MODE = os.environ.get("ATTN_MM_DT", "bf16")

_CACHE = {}


def _build():
    from concourse import bacc
    import concourse.mybir as mybir
    from concourse.tile import TileContext
    from concourse.masks import make_identity

    f32 = mybir.dt.float32
    mdt = mybir.dt.bfloat16 if MM_MODE == "bf16" else mybir.dt.float32r
    FT = mybir.ActivationFunctionType

    nc = bacc.Bacc(trn_type="TRN2", target_bir_lowering=False, debug=False,
                   num_devices=N_CORES)
    xd = nc.dram_tensor("x", [T, C], f32, kind="ExternalInput").ap()
    wqd = nc.dram_tensor("wq", [C, DHG], f32, kind="ExternalInput").ap()
    wkd = nc.dram_tensor("wk", [C, DHG], f32, kind="ExternalInput").ap()
    wvd = nc.dram_tensor("wv", [C, DHG], f32, kind="ExternalInput").ap()
    wod = nc.dram_tensor("wo", [DHG, C], f32, kind="ExternalInput").ap()
    bqd = nc.dram_tensor("bq", [DHG, 1], f32, kind="ExternalInput").ap()
    bkd = nc.dram_tensor("bk", [DHG, 1], f32, kind="ExternalInput").ap()
    bvd = nc.dram_tensor("bv", [1, DHG], f32, kind="ExternalInput").ap()
    mmd = nc.dram_tensor("maskm", [128, 512], f32, kind="ExternalInput").ap()
    yd = nc.dram_tensor("y", [T, C], f32, kind="ExternalOutput").ap()

    with TileContext(nc) as tc:
        with tc.tile_pool(name="persist", bufs=1) as pp:
            qt_all = pp.tile([128, 2 * T], mdt, tag="qt")
            kt_all = pp.tile([128, 2 * T], mdt, tag="kt")
            v_all = pp.tile([128, TB * VW], mdt, tag="v")
            ot_all = pp.tile([128, 2 * T], mdt, tag="ot")
            wo_t = pp.tile([128, 2 * C], mdt, tag="wo")
            maskm = pp.tile([128, 512], f32, tag="mm")
            ident = pp.tile([128, 128], mdt, tag="id")
            ones64 = pp.tile([1, HD], mdt, tag="o64")
            onest = pp.tile([1, 128], mdt, tag="ot1")
            bq_c = pp.tile([128, 2], f32, tag="bqc")
            bk_c = pp.tile([128, 2], f32, tag="bkc")
            bv_r = pp.tile([1, DHG], mdt, tag="bvr")
            zstage = pp.tile([128, 128], f32, tag="zst")
            zrow = pp.tile([128, 384], mdt, tag="zrow")

            nc.sync.dma_start(out=maskm[:], in_=mmd)
            make_identity(nc, ident[:])
            # memset can't produce bf16/f32r reliably across walrus builds;
            # build ones as 0 + 1.0 on ScalarE
            nc.gpsimd.memset(zstage[:], 0.0)
            nc.scalar.add(ones64[:], zstage[0:1, 0:HD], 1.0)
            for j in range(3):
                nc.scalar.add(zrow[:, j * 128:(j + 1) * 128], zstage[:], 0.0)
            nc.scalar.add(onest[:], zstage[0:1, :], 1.0)
            # ones columns for the fused rowsum (strided view, one col per
            # (t-block, head) section); data columns are overwritten by the
            # V-projection copies below.
            v4 = v_all[:].rearrange("p (t h c) -> p t h c", h=HPG, c=HD + 1)
            z4 = zstage[:, 0:TB * HPG].rearrange("p (t h) -> p t h", h=HPG).unsqueeze(3)
            nc.scalar.add(v4[:, :, :, HD:HD + 1], z4, 1.0)
            for pb in range(2):
                nc.sync.dma_start(out=bq_c[:, pb:pb + 1],
                                  in_=bqd[pb * 128:(pb + 1) * 128, :])
                nc.sync.dma_start(out=bk_c[:, pb:pb + 1],
                                  in_=bkd[pb * 128:(pb + 1) * 128, :])
            nc.gpsimd.dma_start(out=bv_r[:], in_=bvd)

            # ---- P1+P2: x load/transpose and Q/K/V projections ----
            with tc.tile_pool(name="ph12", bufs=1) as p12:
                xT = p12.tile([128, CB * T], mdt, tag="xT")
                wq_t = p12.tile([128, CB * DHG], mdt, tag="wqt")
                wk_t = p12.tile([128, CB * DHG], mdt, tag="wkt")
                wv_t = p12.tile([128, CB * DHG], mdt, tag="wvt")
                with (tc.tile_pool(name="xstage", bufs=3) as xs,
                      tc.tile_pool(name="pstr", bufs=4, space="PSUM") as pst):
                    for tb in range(TB):
                        # gpsimd DMA casts fp32 -> mdt on the way in
                        xrow = xs.tile([128, C], mdt)
                        nc.gpsimd.dma_start(out=xrow[:],
                                            in_=xd[tb * 128:(tb + 1) * 128, :])
                        for cb in range(CB):
                            ptr = pst.tile([128, 128], mdt)
                            nc.tensor.transpose(ptr[:], xrow[:, cb * 128:(cb + 1) * 128],
                                                ident[:])
                            dst = xT[:, cb * T + tb * 128: cb * T + (tb + 1) * 128]
                            nc.vector.tensor_copy(out=dst, in_=ptr[:])

                # weights ride the sync (HWDGE) queue in fp32 while x rides
                # gpsimd (casting SWDGE); DVE casts staging -> bf16
                with tc.tile_pool(name="wstage", bufs=1) as ws:
                    wq_s = ws.tile([128, CB * DHG], f32, tag="wqs")
                    wk_s = ws.tile([128, CB * DHG], f32, tag="wks")
                    wv_s = ws.tile([128, CB * DHG], f32, tag="wvs")
                    wo_s = ws.tile([128, C * 2], f32, tag="wos")
                    for cb in range(CB):
                        nc.sync.dma_start(out=wq_s[:, cb * DHG:(cb + 1) * DHG],
                                          in_=wqd[cb * 128:(cb + 1) * 128, :])
                        nc.sync.dma_start(out=wk_s[:, cb * DHG:(cb + 1) * DHG],
                                          in_=wkd[cb * 128:(cb + 1) * 128, :])
                        nc.sync.dma_start(out=wv_s[:, cb * DHG:(cb + 1) * DHG],
                                          in_=wvd[cb * 128:(cb + 1) * 128, :])
                    for pb in range(2):
                        nc.sync.dma_start(out=wo_s[:, pb * C:(pb + 1) * C],
                                          in_=wod[pb * 128:(pb + 1) * 128, :])
                    nc.scalar.copy(wq_t[:], wq_s[:])
                    nc.scalar.copy(wk_t[:], wk_s[:])
                    nc.scalar.copy(wv_t[:], wv_s[:])
                    nc.scalar.copy(wo_t[:], wo_s[:])

                with tc.tile_pool(name="psqk", bufs=3, space="PSUM") as pqk:
                    # tt-outer, V interleaved, so the first q/k/v stripes
                    # finish first and P3 can start while later stripes
                    # are still projecting
                    for tt in range(NQT):
                        for wt, bias_c, dst_all in ((wq_t, bq_c, qt_all),
                                                    (wk_t, bk_c, kt_all)):
                            for pb in range(2):
                                ps = pqk.tile([128, 512], f32, tag="psqk")
                                for cb in range(CB):
                                    nc.tensor.matmul(
                                        ps[:],
                                        wt[:, cb * DHG + pb * 128: cb * DHG + (pb + 1) * 128],
                                        xT[:, cb * T + tt * 512: cb * T + (tt + 1) * 512],
                                        start=(cb == 0), stop=(cb == CB - 1))
                                nc.vector.tensor_scalar_add(
                                    out=dst_all[:, pb * T + tt * 512: pb * T + (tt + 1) * 512],
                                    in0=ps[:], scalar1=bias_c[:, pb:pb + 1])
                        # V projection for this q-tile's t-blocks
                        for tb in range(4 * tt, 4 * tt + 4):
                            psv = pqk.tile([128, DHG], f32, tag="psv")
                            for cb in range(CB):
                                nc.tensor.matmul(
                                    psv[:],
                                    xT[:, cb * T + tb * 128: cb * T + (tb + 1) * 128],
                                    wv_t[:, cb * DHG:(cb + 1) * DHG],
                                    start=(cb == 0), stop=False)
                            nc.tensor.matmul(psv[:], onest[:], bv_r[:],
                                             start=False, stop=True)
                            for hh in range(HPG):
                                dst = v_all[:, tb * VW + hh * (HD + 1):
                                            tb * VW + hh * (HD + 1) + HD]
                                nc.vector.tensor_copy(out=dst,
                                                      in_=psv[:, hh * HD:(hh + 1) * HD])

            # ---- P3: attention; per (q-tile, k-block) all 4 heads are
            # emitted together so PE streams scores (alternating array
            # row-groups) while ACT exps trail one stage behind ----
            with (tc.tile_pool(name="ps_s", bufs=4, space="PSUM") as pss,
                  tc.tile_pool(name="ps_o", bufs=1, space="PSUM") as pso,
                  tc.tile_pool(name="sb3", bufs=6) as sb3):
                for qt in range(NQT):
                    nkb = 4 * (qt + 1)
                    po_l = [pso.tile([128, 512], f32, tag=f"po{h}", name=f"po{h}")
                            for h in range(HPG)]
                    for kb in range(nkb):
                        # cols [0, c) of this k-block's S^T are fully masked
                        # (q < k); skip them in the matmuls and exp, and
                        # zero them in P~ instead
                        c = max(0, (kb - 4 * qt) * 128)
                        sp_l = []
                        for h in range(HPG):
                            pbh, rh = h // 2, 64 * (h % 2)
                            sp = pss.tile([128, 512], f32, tag="sp")
                            nc.tensor.matmul(
                                sp[:, c:512],
                                kt_all[rh:rh + HD, pbh * T + kb * 128: pbh * T + (kb + 1) * 128],
                                qt_all[rh:rh + HD, pbh * T + qt * 512 + c: pbh * T + (qt + 1) * 512],
                                start=True, stop=True)
                            sp_l.append(sp)
                        for h in range(HPG):
                            sp = sp_l[h]
                            if kb >= 4 * qt:
                                nc.vector.tensor_add(out=sp[:, c:c + 128],
                                                     in0=sp[:, c:c + 128],
                                                     in1=maskm[:, 384:512])
                            pt = sb3.tile([128, 512], mdt, tag="pt")
                            if c > 0:
                                nc.vector.tensor_copy(out=pt[:, 0:c], in_=zrow[:, 0:c])
                            nc.scalar.activation(pt[:, c:512], sp[:, c:512], FT.Exp,
                                                 bias=0.0, scale=1.0 / np.sqrt(HD))
                            nc.tensor.matmul(
                                po_l[h][0:HD + 1, c:512],
                                v_all[:, kb * VW + h * (HD + 1): kb * VW + (h + 1) * (HD + 1)],
                                pt[:, c:512], start=(kb == 0), stop=(kb == nkb - 1))
                    for h in range(HPG):
                        pbh, rh = h // 2, 64 * (h % 2)
                        po = po_l[h]
                        rsum = sb3.tile([1, 512], f32, tag="rsum")
                        nc.vector.tensor_copy(out=rsum[:], in_=po[HD:HD + 1, :])
                        rr = sb3.tile([1, 512], f32, tag="rr")
                        nc.vector.reciprocal_approx_fast(out=rr[:], in_=rsum[:])
                        rrm = sb3.tile([1, 512], mdt, tag="rrm")
                        nc.vector.tensor_copy(out=rrm[:], in_=rr[:])
                        # broadcast recip into the unused partitions 64..127
                        # of this head's po bank (rsum row already consumed)
                        nc.tensor.matmul(po[HD:HD + 64, :], ones64[:], rrm[:],
                                         start=True, stop=True,
                                         tile_position=(0, HD))
                        rbs = sb3.tile([HD, 512], f32, tag="rbs")
                        nc.vector.tensor_copy(out=rbs[:], in_=po[HD:HD + 64, :])
                        nc.vector.tensor_mul(
                            out=ot_all[rh:rh + HD, pbh * T + qt * 512: pbh * T + (qt + 1) * 512],
                            in0=po[0:HD, :], in1=rbs[:])

            # ---- P4: output projection (partial; host sums across cores) ----
            with (tc.tile_pool(name="ps_y", bufs=3, space="PSUM") as psy,
                  tc.tile_pool(name="sb4", bufs=3) as sb4):
                for tb in range(TB):
                    for ct in range(2):
                        py = psy.tile([128, 512], f32, tag="py")
                        for pb in range(2):
                            nc.tensor.matmul(
                                py[:],
                                ot_all[:, pb * T + tb * 128: pb * T + (tb + 1) * 128],
                                wo_t[:, pb * C + ct * 512: pb * C + (ct + 1) * 512],
                                start=(pb == 0), stop=(pb == 1))
                        ys = sb4.tile([128, 512], f32, tag="ys")
                        nc.scalar.copy(ys[:], py[:])
                        nc.sync.dma_start(out=yd[tb * 128:(tb + 1) * 128,
                                                 ct * 512:(ct + 1) * 512],
                                          in_=ys[:])
    nc.finalize()
    return nc


def _get_nc():
    if "nc" not in _CACHE:
        _CACHE["nc"] = _build()
    return _CACHE["nc"]


def kernel(x, mask, Wq, bq, Wk, bk, Wv, bv, Wo, bo):
    from concourse import bass_utils

    x = np.ascontiguousarray(np.asarray(x, dtype=np.float32))
    Wq = np.asarray(Wq, dtype=np.float32)
    Wk = np.asarray(Wk, dtype=np.float32)
    Wv = np.asarray(Wv, dtype=np.float32)
    Wo = np.asarray(Wo, dtype=np.float32)
    bq = np.asarray(bq, dtype=np.float32)
    bk = np.asarray(bk, dtype=np.float32)
    bv = np.asarray(bv, dtype=np.float32)
    bo = np.asarray(bo, dtype=np.float32)

    m2 = np.asarray(mask).reshape(T, T)
    if not np.array_equal(m2, np.tril(np.ones((T, T), dtype=bool))):
        raise NotImplementedError("kernel is specialized to the causal tril mask")
    # additive mask master tile in S^T layout [k-row kk, col i]:
    # masked (q < k) iff (i - 384) < kk, where qq = i - (384 - c)
    ii = np.arange(512)[None, :]
    kk = np.arange(128)[:, None]
    maskm = np.ascontiguousarray(
        np.where((ii - 384) < kk, np.float32(NEG), np.float32(0.0)).astype(np.float32))

    nc = _get_nc()
    in_maps = []
    for core in range(N_CORES):
        b, g = core // GROUPS, core % GROUPS
        cols = slice(g * DHG, (g + 1) * DHG)
        in_maps.append({
            "x": np.ascontiguousarray(x[b]),
            "wq": np.ascontiguousarray(Wq[:, cols]),
            "wk": np.ascontiguousarray(Wk[:, cols]),
            "wv": np.ascontiguousarray(Wv[:, cols]),
            "wo": np.ascontiguousarray(Wo[cols, :]),
            "bq": np.ascontiguousarray(bq[cols].reshape(DHG, 1)),
            "bk": np.ascontiguousarray(bk[cols].reshape(DHG, 1)),
            "bv": np.ascontiguousarray(bv[cols].reshape(1, DHG)),
            "maskm": maskm,
        })

    trace = bool(int(os.environ.get("ATTN_TRACE", "0")))
    res = bass_utils.run_bass_kernel_spmd(nc, in_maps,
                                          core_ids=list(range(N_CORES)),
                                          trace=trace)
    _CACHE["last_result"] = res

    out = np.zeros((B, T, C), dtype=np.float32)
    for core in range(N_CORES):
        out[core // GROUPS] += res.results[core]["y"]
    out += bo
    return out
